# revision 27
# baseline (speedup 1.0000x reference)
"""Trainium2 Bass kernel for nn_CSSMUpdateBlock (dense_transformer).

Strategy
--------
* The block has NO cross-token mixing (the CSSM scan runs per track over T,
  LayerNorm/MLP are per token) and the outputs slice away the virtual
  tokens, so the NV=64 virtual tracks are dead computation -> skipped.
* Shard the B*N = 8192 real tracks over 8 cores (1024 tracks / core,
  32768 tokens / core).  No collectives needed.
* On-chip layout: channels on partitions, tokens on the free dim, ordered
  track-major / time-fastest so the CSSM recurrence maps onto the DVE
  `tensor_tensor_scan` instruction.  Track boundaries are handled by
  zeroing the decay column at t==0 of each track (state reset).
* LayerNorm along the partition (channel) axis is done with ones-vector
  matmuls for the stats and broadcast matmuls (lhsT = ln_scale row) for
  the normalization, folding the LN scale into the broadcast.
* The output heads run with the activations as the stationary operand so
  results land in (token, feature) layout for direct DMA out.
* Matmul operands are cast to bf16 (fp32 accumulation in PSUM); the scan
  and other elementwise math stay fp32.
* DMA instruction count is minimized (HWDGE descriptor generation is a
  serialized ~625ns/instruction resource): per 512-token tile, five xbar
  DMA-transposes + one residual load in, one batched store out; constants
  are packed into a handful of tensors loaded once. The 5-wide feature
  tail rides in an overlapping 128-wide transpose chunk whose overlapped
  weight rows are zeroed host-side.
"""

import sys

if "/opt/trn_rl_repo" not in sys.path:
    sys.path.insert(0, "/opt/trn_rl_repo")

import numpy as np
import ml_dtypes

# ----- problem constants (hardcoded, per task statement) -----
B, T, N, D = 4, 32, 2048, 256
DC, DF = 196, 64
H, NV = 256, 64
DIN = D + DC + DF + 1          # 517
F4 = 4 * H                     # 1024
NCORES = 8
NPC = N // NCORES              # 256 n-tracks per core
TPC = B * NPC                  # 1024 tracks per core
TOK = TPC * T                  # 32768 tokens per core
FREE = 512                     # tokens per compute tile (16 tracks * 32 t)
NTILES = TOK // FREE           # 64
GROUPS = FREE // 128           # 4
CHUNKS = [128, 128, 128, 128, 5]   # din k-tiles (sum = 517)
KIN = len(CHUNKS)
NH = H // 128                  # 2 h-tiles
NF = F4 // 128                 # 8 f-tiles
DOUT = 2 + 1 + D               # 259  [delta | vis | track]
EPS = 1e-6

BF16 = ml_dtypes.bfloat16

# Which HW activation to use for gelu. The reference (jax.nn.gelu default)
# is the tanh approximation; CoreSim implements neither gelu (test_sim
# patches one in).
GELU = "Gelu_apprx_tanh"

# matmul-operand dtype: "bfloat16" (fast) or "float32" (debug / precision)
MM_DT = "bfloat16"

# packed col-constant layout: name -> (offset, count)
_COL_ORDER = [("bin", NH), ("bup", NF), ("bdn", NH), ("l1b", NH),
              ("l2b", NH), ("be", NH), ("bi", NH)]
NCOLS = sum(n for _, n in _COL_ORDER)
# packed row-constant layout (bf16): ln1s | ln2s | bcat
NROWS = H + H + DOUT


def _col_off(name):
    o = 0
    for nm, n in _COL_ORDER:
        if nm == name:
            return o
        o += n
    raise KeyError(name)


# ======================================================================
# IO declaration (shared by kernel, sim tests, timeline analysis)
# ======================================================================

def make_io(nc, ntok):
    from concourse import mybir
    bf16 = getattr(mybir.dt, MM_DT)
    f32 = mybir.dt.float32

    def di(name, shape, dtype=f32):
        return nc.dram_tensor(name, list(shape), dtype,
                              kind="ExternalInput").ap()

    def do(name, shape, dtype=f32):
        return nc.dram_tensor(name, list(shape), dtype,
                              kind="ExternalOutput").ap()

    ins = {
        "feats": di("feats", (ntok, DIN), bf16),
        "trackres": di("trackres", (ntok, D)),
        "masks": di("masks", (128, 2 * NH * FREE), bf16),
        "cols": di("cols", (128, NCOLS)),
        "rows": di("rows", (1, NROWS), bf16),
        "win": di("win", (128, KIN * H), bf16),
        "wup": di("wup", (128, NH * F4), bf16),
        "wdn": di("wdn", (128, NF * H), bf16),
        "wcat": di("wcat", (128, NH * DOUT), bf16),
    }
    outs = {
        "ocat": do("ocat", (ntok, DOUT)),
    }
    return ins, outs


# ======================================================================
# Tile kernel builder
# ======================================================================

def build_kernel(tc, outs, ins, ntiles=NTILES, repeat=1):
    """Emit the per-core kernel. outs/ins are dicts of DRAM APs.
    repeat>1 wraps the pipeline in a hardware loop re-running the same
    (idempotent) computation -- used only for timing."""
    from contextlib import ExitStack
    from concourse import mybir

    nc = tc.nc
    f32 = mybir.dt.float32
    bf16 = getattr(mybir.dt, MM_DT)
    AF = mybir.ActivationFunctionType
    OP = mybir.AluOpType

    feats = ins["feats"]
    trackres = ins["trackres"]
    ocat = outs["ocat"]

    ctx = ExitStack()
    with ctx:
        const = ctx.enter_context(tc.tile_pool(name="const", bufs=1))

        # ---- constants (7 batched DMAs + 2 memsets) ----
        ones_h = const.tile([128, 1], bf16, tag="ones_h")
        nc.vector.memset(ones_h[:], 1.0 / H)
        ones_tok = const.tile([1, 128], bf16, tag="ones_tok")
        nc.vector.memset(ones_tok[:], 1.0)

        def load(src_ap, shape, dtype, tag):
            t = const.tile(shape, dtype, tag=tag, name=tag)
            nc.sync.dma_start(out=t[:], in_=src_ap)
            return t

        masks = load(ins["masks"], [128, 2 * NH * FREE], bf16, "masks")
        mE = [masks[:, h * FREE:(h + 1) * FREE] for h in range(NH)]
        mI = [masks[:, (NH + h) * FREE:(NH + h + 1) * FREE] for h in range(NH)]

        cols = load(ins["cols"], [128, NCOLS], f32, "cols")

        def col(name, j):
            o = _col_off(name) + j
            return cols[:, o:o + 1]

        rows = load(ins["rows"], [1, NROWS], bf16, "rows")
        ln1s = [rows[:, h * 128:(h + 1) * 128] for h in range(NH)]
        ln2s = [rows[:, H + h * 128:H + (h + 1) * 128] for h in range(NH)]
        bcat = rows[:, 2 * H:2 * H + DOUT]

        winb = load(ins["win"], [128, KIN * H], bf16, "winb")

        def w_in(k, h):
            return winb[:, k * H + h * 128:k * H + (h + 1) * 128]

        wupb = load(ins["wup"], [128, NH * F4], bf16, "wupb")

        def w_up(hk, f):
            return wupb[:, hk * F4 + f * 128:hk * F4 + (f + 1) * 128]

        wdnb = load(ins["wdn"], [128, NF * H], bf16, "wdnb")

        def w_dn(fk, h):
            return wdnb[:, fk * H + h * 128:fk * H + (h + 1) * 128]

        wcatb = load(ins["wcat"], [128, NH * DOUT], bf16, "wcatb")

        def w_cat(hk):
            return wcatb[:, hk * DOUT:(hk + 1) * DOUT]

        # ---- pools ----
        # NOTE: bufs is per-tag; pools with per-h/f tags use bufs=2
        trk_p = ctx.enter_context(tc.tile_pool(name="trk", bufs=2))
        featT = ctx.enter_context(tc.tile_pool(name="featT", bufs=2 * KIN))
        xh_p = ctx.enter_context(tc.tile_pool(name="xh", bufs=2))
        scan_p = ctx.enter_context(tc.tile_pool(name="scan", bufs=2))
        r_p = ctx.enter_context(tc.tile_pool(name="r", bufs=2))
        sq_p = ctx.enter_context(tc.tile_pool(name="sq", bufs=2))
        row_p = ctx.enter_context(tc.tile_pool(name="rows_p", bufs=6))
        z_p = ctx.enter_context(tc.tile_pool(name="z", bufs=2))
        xv_p = ctx.enter_context(tc.tile_pool(name="xv", bufs=2))
        gup_p = ctx.enter_context(tc.tile_pool(name="gup", bufs=2))
        xv2_p = ctx.enter_context(tc.tile_pool(name="xv2", bufs=2))
        out_p = ctx.enter_context(tc.tile_pool(name="out", bufs=2))

        ps = ctx.enter_context(tc.tile_pool(name="ps", bufs=1, space="PSUM"))

        def psum(tag, bufs, shape=(128, FREE), dtype=None):
            return ps.tile(list(shape), dtype or f32, tag=tag, bufs=bufs,
                           name=tag)

        def layer_norm(r, s_rows, bname, out_pool, tag):
            """r: list of NH (128, FREE) bf16 tiles. Returns NH bf16 tiles."""
            Sp = psum("sh", 2)[:1, :]
            for h in range(NH):
                nc.tensor.matmul(Sp[:], lhsT=ones_h[:], rhs=r[h][:],
                                 start=(h == 0), stop=(h == NH - 1))
            Qp = psum("sh", 2)[:1, :]
            for h in range(NH):
                sq = sq_p.tile([128, FREE], bf16, tag="sq")
                nc.scalar.activation(sq[:], r[h][:], AF.Square)
                nc.tensor.matmul(Qp[:], lhsT=ones_h[:], rhs=sq[:],
                                 start=(h == 0), stop=(h == NH - 1))
            # var + eps = (Q + eps) - S^2     (S, Q already scaled by 1/H)
            t1 = row_p.tile([1, FREE], f32, tag="lnrow")
            nc.scalar.activation(t1[:], Sp[:], AF.Square)
            v = row_p.tile([1, FREE], f32, tag="lnrow")
            nc.vector.scalar_tensor_tensor(v[:], Qp[:], EPS, t1[:],
                                           op0=OP.add, op1=OP.subtract)
            rec = row_p.tile([1, FREE], f32, tag="lnrow")
            nc.vector.reciprocal(rec[:], v[:])
            rs = row_p.tile([1, FREE], bf16, tag="lnrowb")
            nc.scalar.activation(rs[:], rec[:], AF.Sqrt)
            p = row_p.tile([1, FREE], bf16, tag="lnrowb")
            nc.vector.tensor_tensor(p[:], Sp[:], rs[:], op=OP.mult)

            out = []
            for h in range(NH):
                bc_rs = psum("bcast", 2)
                nc.tensor.matmul(bc_rs[:], lhsT=s_rows[h], rhs=rs[:],
                                 start=True, stop=True)
                bc_p = psum("bcast", 2)
                nc.tensor.matmul(bc_p[:], lhsT=s_rows[h], rhs=p[:],
                                 start=True, stop=True)
                z = z_p.tile([128, FREE], f32, tag="z")
                nc.vector.tensor_tensor(z[:], r[h][:], bc_rs[:], op=OP.mult)
                o = out_pool.tile([128, FREE], bf16, tag=f"{tag}{h}")
                nc.vector.scalar_tensor_tensor(o[:], z[:], col(bname, h),
                                               bc_p[:], op0=OP.add,
                                               op1=OP.subtract)
                out.append(o)
            return out

        TAIL = CHUNKS[4]  # 5

        def stage_a_load(it):
            """Load, transpose, W_in matmul + gelu -> (xh, trk)."""
            tok0 = it * FREE

            # ---- residual input (fp32) ----
            trk = trk_p.tile([128, GROUPS * D], f32, tag="trk")
            nc.sync.dma_start(
                out=trk[:].rearrange("p (g d) -> p g d", d=D),
                in_=trackres[tok0:tok0 + FREE, :].rearrange(
                    "(g p) d -> p g d", p=128))

            # ---- (din, token) tiles via xbar DMA-transpose. The 5-wide
            # tail [512:517) rides in a 5th chunk at offset 389 that
            # OVERLAPS chunk 3; the weight rows for the overlapped span
            # [389:512) are zeroed host-side so the contraction is exact. --
            fT = []
            for c in range(KIN):
                o = c * 128 if c < 4 else DIN - 128     # 389
                f = featT.tile([128, FREE], bf16, tag="fT")
                nc.sync.dma_start(
                    out=f[:], in_=feats[tok0:tok0 + FREE, o:o + 128],
                    transpose=True)
                fT.append(f)

            # ---- x = gelu(feats @ W_in + b_in) ----
            xh = []
            for h in range(NH):
                pw = psum("wmm", 2)
                for k in range(KIN):
                    nc.tensor.matmul(pw[:], lhsT=w_in(k, h), rhs=fT[k][:],
                                     start=(k == 0), stop=(k == KIN - 1))
                x = xh_p.tile([128, FREE], bf16, tag=f"xh{h}")
                nc.scalar.activation(x[:], pw[:], getattr(AF, GELU),
                                     bias=col("bin", h))
                xh.append(x)
            return xh, trk

        def stage_a_scan(xh):
            """CSSM opponent scan over t (free dim), r = x + (e - i)."""
            r = []
            for h in range(NH):
                s1 = scan_p.tile([128, FREE], bf16, tag="s1")
                nc.vector.tensor_scalar_mul(s1[:], xh[h][:], col("be", h))
                e = scan_p.tile([128, FREE], bf16, tag="e")
                nc.vector.tensor_tensor_scan(e[:], mE[h], s1[:], 0.0,
                                             op0=OP.mult, op1=OP.add)
                s2 = scan_p.tile([128, FREE], bf16, tag="s2")
                nc.vector.tensor_scalar_mul(s2[:], e[:], col("bi", h))
                i_t = scan_p.tile([128, FREE], bf16, tag="i")
                nc.vector.tensor_tensor_scan(i_t[:], mI[h], s2[:], 0.0,
                                             op0=OP.mult, op1=OP.add)
                y = scan_p.tile([128, FREE], bf16, tag="y")
                nc.vector.tensor_tensor(y[:], e[:], i_t[:], op=OP.subtract)
                rr = r_p.tile([128, FREE], bf16, tag=f"r{h}")
                nc.vector.tensor_tensor(rr[:], xh[h][:], y[:], op=OP.add)
                r.append(rr)
            return r

        def stage_b1(r):
            """LN1."""
            return layer_norm(r, ln1s, "l1b", xv_p, "xv")

        def stage_b2(it, xv, trk):
            """MLP, LN2, heads, store."""
            tok0 = it * FREE

            # ---- MLP up: gelu(xv @ W_up + b_up) ----
            gup = []
            for f in range(NF):
                pu = psum("mm", 2)
                for hk in range(NH):
                    nc.tensor.matmul(pu[:], lhsT=w_up(hk, f), rhs=xv[hk][:],
                                     start=(hk == 0), stop=(hk == NH - 1))
                g = gup_p.tile([128, FREE], bf16, tag=f"g{f}")
                nc.scalar.activation(g[:], pu[:], getattr(AF, GELU),
                                     bias=col("bup", f))
                gup.append(g)

            # ---- MLP down + residual:  u = xv + (h @ W_dn + b_dn) ----
            u = []
            for h in range(NH):
                pd = psum("mm", 2)
                for fk in range(NF):
                    nc.tensor.matmul(pd[:], lhsT=w_dn(fk, h), rhs=gup[fk][:],
                                     start=(fk == 0), stop=(fk == NF - 1))
                uu = xv2_p.tile([128, FREE], bf16, tag=f"u{h}")
                nc.vector.scalar_tensor_tensor(uu[:], pd[:], col("bdn", h),
                                               xv[h][:], op0=OP.add,
                                               op1=OP.add)
                u.append(uu)

            # ---- LN2 ----
            xv2 = layer_norm(u, ln2s, "l2b", xv2_p, "xv2")

            # ---- heads: [delta|vis|track] = xv2 @ Wcat + bcat (+ residual) --
            oc = out_p.tile([128, GROUPS * DOUT], f32, tag="oc")
            for g in range(GROUPS):
                ph = psum("sh", 2)[:, :DOUT]
                for hk in range(NH):
                    nc.tensor.matmul(ph[:],
                                     lhsT=xv2[hk][:, g * 128:(g + 1) * 128],
                                     rhs=w_cat(hk),
                                     start=(hk == 0), stop=False)
                nc.tensor.matmul(ph[:], lhsT=ones_tok[:], rhs=bcat,
                                 start=False, stop=True)
                nc.vector.tensor_tensor(
                    oc[:, g * DOUT + 3:(g + 1) * DOUT], ph[:, 3:3 + D],
                    trk[:, g * D:(g + 1) * D], op=OP.add)
                nc.scalar.activation(oc[:, g * DOUT:g * DOUT + 3], ph[:, 0:3],
                                     AF.Copy)
            nc.gpsimd.dma_start(
                out=ocat[tok0:tok0 + FREE, :].rearrange(
                    "(g p) c -> p g c", p=128),
                in_=oc[:].rearrange("p (g c) -> p g c", c=DOUT))

        # software pipeline, interleaved so each engine's in-order queue
        # matches data readiness: tile it+1's loads/W_in go out early, its
        # DVE scan chain is emitted BETWEEN tile it's LN1 and MLP (it fills
        # the DVE idle window while PE/ACT run tile it's MLP), and tile
        # it's LN1 combine is not queued behind a stalled scan.
        def pipeline():
            xh0, trk0 = stage_a_load(0)
            pend_r = stage_a_scan(xh0)
            pend_trk = trk0
            for it in range(ntiles):
                nxt = stage_a_load(it + 1) if it + 1 < ntiles else None
                xv = stage_b1(pend_r)
                nxt_r = stage_a_scan(nxt[0]) if nxt else None
                stage_b2(it, xv, pend_trk)
                pend_r = nxt_r
                pend_trk = nxt[1] if nxt else None

        if repeat == 1:
            pipeline()
        else:
            with tc.For_i(0, repeat, 1):
                pipeline()


# ======================================================================
# Host-side input prep
# ======================================================================

def _sigmoid(x):
    return 1.0 / (1.0 + np.exp(-x))


def prep_params(inputs):
    """Small-parameter preprocessing shared by every core. Returns dict of
    numpy arrays keyed by kernel input name (excluding feats)."""
    f32 = np.float32
    a_e = _sigmoid(inputs["decay_e"].astype(np.float64)).astype(f32)
    a_i = _sigmoid(inputs["decay_i"].astype(np.float64)).astype(f32)
    b_e = ((1.0 - a_e) * inputs["w_e"]).astype(f32)
    b_i = ((1.0 - a_i) * inputs["w_i"]).astype(f32)

    # scan decay tiles with zeros at t==0 of each track (state reset)
    j = np.arange(FREE)
    boundary = (j % T == 0)
    mE = np.where(boundary[None, :], 0.0, a_e[:, None]).astype(f32)
    mI = np.where(boundary[None, :], 0.0, a_i[:, None]).astype(f32)
    masks = np.empty((128, 2 * NH * FREE), f32)  # cast below
    for h in range(NH):
        masks[:, h * FREE:(h + 1) * FREE] = mE[h * 128:(h + 1) * 128]
        masks[:, (NH + h) * FREE:(NH + h + 1) * FREE] = \
            mI[h * 128:(h + 1) * 128]

    cols = np.zeros((128, NCOLS), f32)

    def put_cols(name, vec, n):
        o = _col_off(name)
        for jj in range(n):
            cols[:, o + jj] = vec[jj * 128:(jj + 1) * 128]

    put_cols("bin", inputs["b_in"], NH)
    put_cols("bup", inputs["b_up"], NF)
    put_cols("bdn", inputs["b_dn"], NH)
    put_cols("l1b", inputs["ln1_b"], NH)
    put_cols("l2b", inputs["ln2_b"], NH)
    put_cols("be", b_e, NH)
    put_cols("bi", b_i, NH)

    bcat = np.concatenate(
        [inputs["b_delta"], inputs["b_vis"], inputs["b_track"]], axis=0)
    rows = np.concatenate(
        [inputs["ln1_s"], inputs["ln2_s"], bcat]).reshape(1, NROWS)

    def pack_win(w):
        # chunks at offsets [0,128,256,384,389]; rows [389:512) of the tail
        # chunk are zeroed (they overlap chunk 3)
        tail = np.zeros((128, H), w.dtype)
        tail[123:, :] = w[512:DIN, :]
        stack = [w[c * 128:(c + 1) * 128, :] for c in range(4)] + [tail]
        return np.ascontiguousarray(
            np.stack(stack, 1).reshape(128, KIN * H))

    def pack_k(w, nk):
        # (nk*128, M) -> (128, nk*M)
        M = w.shape[1]
        return np.ascontiguousarray(
            w.reshape(nk, 128, M).transpose(1, 0, 2).reshape(128, nk * M))

    wcat = np.concatenate(
        [inputs["W_delta"], inputs["W_vis"], inputs["W_track"]], axis=1)

    return {
        "masks": masks.astype(BF16),
        "cols": cols,
        "rows": rows.astype(BF16),
        "win": pack_win(inputs["W_in"]).astype(BF16),
        "wup": pack_k(inputs["W_up"], NH).astype(BF16),
        "wdn": pack_k(inputs["W_dn"], NF).astype(BF16),
        "wcat": pack_k(wcat, NH).astype(BF16),
    }


def prep_core_feats(inputs, c):
    """Per-core feature tensors in track-major / time-fastest token order.
    Returns (feats_bf16, track_f32)."""
    sl = slice(c * NPC, (c + 1) * NPC)

    def perm(x):
        return np.ascontiguousarray(
            x[:, :, sl, :].transpose(0, 2, 1, 3)).reshape(TOK, -1)

    track = perm(inputs["track_feats"])
    feats = np.empty((TOK, DIN), BF16)
    feats[:, 0:D] = track
    feats[:, D:D + DC] = perm(inputs["corr_feats"])
    feats[:, D + DC:D + DC + DF] = perm(inputs["flow_feats"])
    feats[:, D + DC + DF:DIN] = perm(inputs["vis"])
    return feats, np.ascontiguousarray(track, dtype=np.float32)


def unperm_core(out_c, nfeat):
    """(TOK, nfeat) track-major -> (B, T, NPC, nfeat)."""
    return out_c.reshape(B, NPC, T, nfeat).transpose(0, 2, 1, 3)


# ======================================================================
# Entry point
# ======================================================================

_CACHE = {}


def _build_nc():
    import concourse.bacc as bacc
    import concourse.tile as tile

    if "nc" in _CACHE:
        return _CACHE["nc"]

    nc = bacc.Bacc("TRN2", target_bir_lowering=False, debug=False)
    ins, outs = make_io(nc, TOK)
    with tile.TileContext(nc) as tc:
        build_kernel(tc, outs, ins, ntiles=NTILES)
    nc.compile()
    _CACHE["nc"] = nc
    return nc


def kernel(**inputs):
    from concourse.bass_utils import run_bass_kernel_spmd

    inputs = {k: np.asarray(v) for k, v in inputs.items()}
    params = prep_params(inputs)

    in_maps = []
    for c in range(NCORES):
        m = dict(params)
        m["feats"], m["trackres"] = prep_core_feats(inputs, c)
        in_maps.append(m)

    nc = _build_nc()
    res = run_bass_kernel_spmd(nc, in_maps, core_ids=list(range(NCORES)))

    delta = np.empty((B, T, N, 2), np.float32)
    vis_o = np.empty((B, T, N, 1), np.float32)
    track_o = np.empty((B, T, N, D), np.float32)
    for c in range(NCORES):
        sl = slice(c * NPC, (c + 1) * NPC)
        r = res.results[c]
        oc = unperm_core(r["ocat"], DOUT)
        delta[:, :, sl, :] = oc[:, :, :, 0:2]
        vis_o[:, :, sl, :] = oc[:, :, :, 2:3]
        track_o[:, :, sl, :] = oc[:, :, :, 3:]
    return (delta, vis_o, track_o)


# revision 28
# speedup vs baseline: 1.2129x; 1.2129x over previous
"""Trainium2 Bass kernel for nn_CSSMUpdateBlock (dense_transformer).

Strategy
--------
* The block has NO cross-token mixing (the CSSM scan runs per track over T,
  LayerNorm/MLP are per token) and the outputs slice away the virtual
  tokens, so the NV=64 virtual tracks are dead computation -> skipped.
* Shard the B*N = 8192 real tracks over 8 cores (1024 tracks / core,
  32768 tokens / core).  No collectives needed.
* On-chip layout: channels on partitions, tokens on the free dim, ordered
  track-major / time-fastest so the CSSM recurrence maps onto the DVE
  `tensor_tensor_scan` instruction.  Track boundaries are handled by
  zeroing the decay column at t==0 of each track (state reset).
* LayerNorm along the partition (channel) axis is done with ones-vector
  matmuls for the stats and broadcast matmuls (lhsT = ln_scale row) for
  the normalization, folding the LN scale into the broadcast.
* The output heads run with the activations as the stationary operand so
  results land in (token, feature) layout for direct DMA out.
* Matmul operands are cast to bf16 (fp32 accumulation in PSUM); the scan
  and other elementwise math stay fp32.
* DMA instruction count is minimized (HWDGE descriptor generation is a
  serialized ~625ns/instruction resource): per 512-token tile, five xbar
  DMA-transposes + one residual load in, one batched store out; constants
  are packed into a handful of tensors loaded once. The 5-wide feature
  tail rides in an overlapping 128-wide transpose chunk whose overlapped
  weight rows are zeroed host-side.
"""

import sys

if "/opt/trn_rl_repo" not in sys.path:
    sys.path.insert(0, "/opt/trn_rl_repo")

import numpy as np
import ml_dtypes

# ----- problem constants (hardcoded, per task statement) -----
B, T, N, D = 4, 32, 2048, 256
DC, DF = 196, 64
H, NV = 256, 64
DIN = D + DC + DF + 1          # 517
F4 = 4 * H                     # 1024
NCORES = 8
NPC = N // NCORES              # 256 n-tracks per core
TPC = B * NPC                  # 1024 tracks per core
TOK = TPC * T                  # 32768 tokens per core
FREE = 512                     # tokens per compute tile (16 tracks * 32 t)
NTILES = TOK // FREE           # 64
GROUPS = FREE // 128           # 4
CHUNKS = [128, 128, 128, 128, 5]   # din k-tiles (sum = 517)
KIN = len(CHUNKS)
NH = H // 128                  # 2 h-tiles
NF = F4 // 128                 # 8 f-tiles
DOUT = 2 + 1 + D               # 259  [delta | vis | track]
EPS = 1e-6

BF16 = ml_dtypes.bfloat16

# Which HW activation to use for gelu. The reference (jax.nn.gelu default)
# is the tanh approximation; CoreSim implements neither gelu (test_sim
# patches one in).
GELU = "Gelu_apprx_tanh"

# matmul-operand dtype: "bfloat16" (fast) or "float32" (debug / precision)
MM_DT = "bfloat16"

# packed col-constant layout: name -> (offset, count)
_COL_ORDER = [("bin", NH), ("bup", NF), ("bdn", NH), ("l1b", NH),
              ("l2b", NH), ("be", NH), ("bi", NH)]
NCOLS = sum(n for _, n in _COL_ORDER)
# packed row-constant layout (bf16): ln1s | ln2s | bcat
NROWS = H + H + DOUT


def _col_off(name):
    o = 0
    for nm, n in _COL_ORDER:
        if nm == name:
            return o
        o += n
    raise KeyError(name)


# ======================================================================
# IO declaration (shared by kernel, sim tests, timeline analysis)
# ======================================================================

def make_io(nc, ntok):
    from concourse import mybir
    bf16 = getattr(mybir.dt, MM_DT)
    f32 = mybir.dt.float32

    def di(name, shape, dtype=f32):
        return nc.dram_tensor(name, list(shape), dtype,
                              kind="ExternalInput").ap()

    def do(name, shape, dtype=f32):
        return nc.dram_tensor(name, list(shape), dtype,
                              kind="ExternalOutput").ap()

    ins = {
        "feats": di("feats", (ntok, DIN), bf16),
        "trackres": di("trackres", (ntok, D)),
        "masks": di("masks", (128, 2 * NH * FREE), bf16),
        "cols": di("cols", (128, NCOLS)),
        "rows": di("rows", (1, NROWS), bf16),
        "win": di("win", (128, KIN * H), bf16),
        "wup": di("wup", (128, NH * F4), bf16),
        "wdn": di("wdn", (128, NF * H), bf16),
        "wcat": di("wcat", (128, NH * DOUT), bf16),
    }
    outs = {
        "ocat": do("ocat", (ntok, DOUT)),
    }
    return ins, outs


# ======================================================================
# Tile kernel builder
# ======================================================================

def build_kernel(tc, outs, ins, ntiles=NTILES, repeat=1):
    """Emit the per-core kernel. outs/ins are dicts of DRAM APs.
    repeat>1 wraps the pipeline in a hardware loop re-running the same
    (idempotent) computation -- used only for timing."""
    from contextlib import ExitStack
    from concourse import mybir

    nc = tc.nc
    f32 = mybir.dt.float32
    bf16 = getattr(mybir.dt, MM_DT)
    AF = mybir.ActivationFunctionType
    OP = mybir.AluOpType

    feats = ins["feats"]
    trackres = ins["trackres"]
    ocat = outs["ocat"]

    ctx = ExitStack()
    with ctx:
        const = ctx.enter_context(tc.tile_pool(name="const", bufs=1))

        # ---- constants (7 batched DMAs + 2 memsets) ----
        ones_h = const.tile([128, 1], bf16, tag="ones_h")
        nc.vector.memset(ones_h[:], 1.0 / H)
        ones_tok = const.tile([1, 128], bf16, tag="ones_tok")
        nc.vector.memset(ones_tok[:], 1.0)

        def load(src_ap, shape, dtype, tag):
            t = const.tile(shape, dtype, tag=tag, name=tag)
            nc.sync.dma_start(out=t[:], in_=src_ap)
            return t

        masks = load(ins["masks"], [128, 2 * NH * FREE], bf16, "masks")
        mE = [masks[:, h * FREE:(h + 1) * FREE] for h in range(NH)]
        mI = [masks[:, (NH + h) * FREE:(NH + h + 1) * FREE] for h in range(NH)]

        cols = load(ins["cols"], [128, NCOLS], f32, "cols")

        def col(name, j):
            o = _col_off(name) + j
            return cols[:, o:o + 1]

        rows = load(ins["rows"], [1, NROWS], bf16, "rows")
        ln1s = [rows[:, h * 128:(h + 1) * 128] for h in range(NH)]
        ln2s = [rows[:, H + h * 128:H + (h + 1) * 128] for h in range(NH)]
        bcat = rows[:, 2 * H:2 * H + DOUT]

        winb = load(ins["win"], [128, KIN * H], bf16, "winb")

        def w_in(k, h):
            return winb[:, k * H + h * 128:k * H + (h + 1) * 128]

        wupb = load(ins["wup"], [128, NH * F4], bf16, "wupb")

        def w_up(hk, f):
            return wupb[:, hk * F4 + f * 128:hk * F4 + (f + 1) * 128]

        wdnb = load(ins["wdn"], [128, NF * H], bf16, "wdnb")

        def w_dn(fk, h):
            return wdnb[:, fk * H + h * 128:fk * H + (h + 1) * 128]

        wcatb = load(ins["wcat"], [128, NH * DOUT], bf16, "wcatb")

        def w_cat(hk):
            return wcatb[:, hk * DOUT:(hk + 1) * DOUT]

        # ---- pools ----
        # NOTE: bufs is per-tag; pools with per-h/f tags use bufs=2
        trk_p = ctx.enter_context(tc.tile_pool(name="trk", bufs=2))
        featT = ctx.enter_context(tc.tile_pool(name="featT", bufs=2 * KIN))
        xh_p = ctx.enter_context(tc.tile_pool(name="xh", bufs=2))
        scan_p = ctx.enter_context(tc.tile_pool(name="scan", bufs=2))
        r_p = ctx.enter_context(tc.tile_pool(name="r", bufs=2))
        sq_p = ctx.enter_context(tc.tile_pool(name="sq", bufs=2))
        row_p = ctx.enter_context(tc.tile_pool(name="rows_p", bufs=6))
        z_p = ctx.enter_context(tc.tile_pool(name="z", bufs=2))
        xv_p = ctx.enter_context(tc.tile_pool(name="xv", bufs=2))
        gup_p = ctx.enter_context(tc.tile_pool(name="gup", bufs=2))
        xv2_p = ctx.enter_context(tc.tile_pool(name="xv2", bufs=2))
        out_p = ctx.enter_context(tc.tile_pool(name="out", bufs=2))

        ps = ctx.enter_context(tc.tile_pool(name="ps", bufs=1, space="PSUM"))

        def psum(tag, bufs, shape=(128, FREE), dtype=None):
            return ps.tile(list(shape), dtype or f32, tag=tag, bufs=bufs,
                           name=tag)

        def layer_norm(r, s_rows, bname, out_pool, tag):
            """r: list of NH (128, FREE) bf16 tiles. Returns NH bf16 tiles."""
            Sp = psum("sh", 2)[:1, :]
            for h in range(NH):
                nc.tensor.matmul(Sp[:], lhsT=ones_h[:], rhs=r[h][:],
                                 start=(h == 0), stop=(h == NH - 1))
            Qp = psum("sh", 2)[:1, :]
            for h in range(NH):
                sq = sq_p.tile([128, FREE], bf16, tag="sq")
                nc.scalar.activation(sq[:], r[h][:], AF.Square)
                nc.tensor.matmul(Qp[:], lhsT=ones_h[:], rhs=sq[:],
                                 start=(h == 0), stop=(h == NH - 1))
            # var + eps = (Q + eps) - S^2     (S, Q already scaled by 1/H)
            t1 = row_p.tile([1, FREE], f32, tag="lnrow")
            nc.scalar.activation(t1[:], Sp[:], AF.Square)
            v = row_p.tile([1, FREE], f32, tag="lnrow")
            nc.vector.scalar_tensor_tensor(v[:], Qp[:], EPS, t1[:],
                                           op0=OP.add, op1=OP.subtract)
            rec = row_p.tile([1, FREE], f32, tag="lnrow")
            nc.vector.reciprocal(rec[:], v[:])
            rs = row_p.tile([1, FREE], bf16, tag="lnrowb")
            nc.scalar.activation(rs[:], rec[:], AF.Sqrt)
            p = row_p.tile([1, FREE], bf16, tag="lnrowb")
            nc.vector.tensor_tensor(p[:], Sp[:], rs[:], op=OP.mult)

            out = []
            for h in range(NH):
                bc_rs = psum("bcast", 2)
                nc.tensor.matmul(bc_rs[:], lhsT=s_rows[h], rhs=rs[:],
                                 start=True, stop=True)
                bc_p = psum("bcast", 2)
                nc.tensor.matmul(bc_p[:], lhsT=s_rows[h], rhs=p[:],
                                 start=True, stop=True)
                z = z_p.tile([128, FREE], f32, tag="z")
                nc.vector.tensor_tensor(z[:], r[h][:], bc_rs[:], op=OP.mult)
                o = out_pool.tile([128, FREE], bf16, tag=f"{tag}{h}")
                nc.vector.scalar_tensor_tensor(o[:], z[:], col(bname, h),
                                               bc_p[:], op0=OP.add,
                                               op1=OP.subtract)
                out.append(o)
            return out

        TAIL = CHUNKS[4]  # 5

        def stage_a_load(it):
            """Load, transpose, W_in matmul + gelu -> (xh, trk)."""
            tok0 = it * FREE

            # ---- residual input (fp32) ----
            trk = trk_p.tile([128, GROUPS * D], f32, tag="trk")
            nc.sync.dma_start(
                out=trk[:].rearrange("p (g d) -> p g d", d=D),
                in_=trackres[tok0:tok0 + FREE, :].rearrange(
                    "(g p) d -> p g d", p=128))

            # ---- (din, token) tiles via xbar DMA-transpose. The 5-wide
            # tail [512:517) rides in a 5th chunk at offset 389 that
            # OVERLAPS chunk 3; the weight rows for the overlapped span
            # [389:512) are zeroed host-side so the contraction is exact. --
            fT = []
            for c in range(KIN):
                o = c * 128 if c < 4 else DIN - 128     # 389
                f = featT.tile([128, FREE], bf16, tag="fT")
                nc.sync.dma_start(
                    out=f[:], in_=feats[tok0:tok0 + FREE, o:o + 128],
                    transpose=True)
                fT.append(f)

            # ---- x = gelu(feats @ W_in + b_in) ----
            xh = []
            for h in range(NH):
                pw = psum("wmm", 1)
                for k in range(KIN):
                    nc.tensor.matmul(pw[:], lhsT=w_in(k, h), rhs=fT[k][:],
                                     start=(k == 0), stop=(k == KIN - 1))
                x = xh_p.tile([128, FREE], bf16, tag=f"xh{h}")
                nc.scalar.activation(x[:], pw[:], getattr(AF, GELU),
                                     bias=col("bin", h))
                xh.append(x)
            return xh, trk

        def stage_a_scan(xh):
            """CSSM opponent scan over t (free dim), r = x + (e - i)."""
            r = []
            for h in range(NH):
                s1 = scan_p.tile([128, FREE], bf16, tag="s1")
                nc.vector.tensor_scalar_mul(s1[:], xh[h][:], col("be", h))
                e = scan_p.tile([128, FREE], bf16, tag="e")
                nc.vector.tensor_tensor_scan(e[:], mE[h], s1[:], 0.0,
                                             op0=OP.mult, op1=OP.add)
                s2 = scan_p.tile([128, FREE], bf16, tag="s2")
                nc.vector.tensor_scalar_mul(s2[:], e[:], col("bi", h))
                i_t = scan_p.tile([128, FREE], bf16, tag="i")
                nc.vector.tensor_tensor_scan(i_t[:], mI[h], s2[:], 0.0,
                                             op0=OP.mult, op1=OP.add)
                y = scan_p.tile([128, FREE], bf16, tag="y")
                nc.vector.tensor_tensor(y[:], e[:], i_t[:], op=OP.subtract)
                rr = r_p.tile([128, FREE], bf16, tag=f"r{h}")
                nc.vector.tensor_tensor(rr[:], xh[h][:], y[:], op=OP.add)
                r.append(rr)
            return r

        def stage_b1(r):
            """LN1."""
            return layer_norm(r, ln1s, "l1b", xv_p, "xv")

        def stage_b2(it, xv, trk):
            """MLP, LN2, heads, store."""
            tok0 = it * FREE

            # ---- MLP up: gelu(xv @ W_up + b_up) ----
            gup = []
            for f in range(NF):
                pu = psum("mm", 3)
                for hk in range(NH):
                    nc.tensor.matmul(pu[:], lhsT=w_up(hk, f), rhs=xv[hk][:],
                                     start=(hk == 0), stop=(hk == NH - 1))
                g = gup_p.tile([128, FREE], bf16, tag=f"g{f}")
                nc.scalar.activation(g[:], pu[:], getattr(AF, GELU),
                                     bias=col("bup", f))
                gup.append(g)

            # ---- MLP down + residual:  u = xv + (h @ W_dn + b_dn) ----
            u = []
            for h in range(NH):
                pd = psum("mm", 3)
                for fk in range(NF):
                    nc.tensor.matmul(pd[:], lhsT=w_dn(fk, h), rhs=gup[fk][:],
                                     start=(fk == 0), stop=(fk == NF - 1))
                uu = xv2_p.tile([128, FREE], bf16, tag=f"u{h}")
                nc.vector.scalar_tensor_tensor(uu[:], pd[:], col("bdn", h),
                                               xv[h][:], op0=OP.add,
                                               op1=OP.add)
                u.append(uu)

            # ---- LN2 ----
            xv2 = layer_norm(u, ln2s, "l2b", xv2_p, "xv2")

            # ---- heads: [delta|vis|track] = xv2 @ Wcat + bcat (+ residual) --
            oc = out_p.tile([128, GROUPS * DOUT], f32, tag="oc")
            for g in range(GROUPS):
                ph = psum("sh", 2)[:, :DOUT]
                for hk in range(NH):
                    nc.tensor.matmul(ph[:],
                                     lhsT=xv2[hk][:, g * 128:(g + 1) * 128],
                                     rhs=w_cat(hk),
                                     start=(hk == 0), stop=False)
                nc.tensor.matmul(ph[:], lhsT=ones_tok[:], rhs=bcat,
                                 start=False, stop=True)
                nc.vector.tensor_tensor(
                    oc[:, g * DOUT + 3:(g + 1) * DOUT], ph[:, 3:3 + D],
                    trk[:, g * D:(g + 1) * D], op=OP.add)
                nc.scalar.activation(oc[:, g * DOUT:g * DOUT + 3], ph[:, 0:3],
                                     AF.Copy)
            nc.gpsimd.dma_start(
                out=ocat[tok0:tok0 + FREE, :].rearrange(
                    "(g p) c -> p g c", p=128),
                in_=oc[:].rearrange("p (g c) -> p g c", c=DOUT))

        # software pipeline, interleaved so each engine's in-order queue
        # matches data readiness: tile it+1's loads/W_in go out early, its
        # DVE scan chain is emitted BETWEEN tile it's LN1 and MLP (it fills
        # the DVE idle window while PE/ACT run tile it's MLP), and tile
        # it's LN1 combine is not queued behind a stalled scan.
        def pipeline():
            xh0, trk0 = stage_a_load(0)
            pend_r = stage_a_scan(xh0)
            pend_trk = trk0
            for it in range(ntiles):
                nxt = stage_a_load(it + 1) if it + 1 < ntiles else None
                xv = stage_b1(pend_r)
                nxt_r = stage_a_scan(nxt[0]) if nxt else None
                stage_b2(it, xv, pend_trk)
                pend_r = nxt_r
                pend_trk = nxt[1] if nxt else None

        if repeat == 1:
            pipeline()
        else:
            with tc.For_i(0, repeat, 1):
                pipeline()


# ======================================================================
# Host-side input prep
# ======================================================================

def _sigmoid(x):
    return 1.0 / (1.0 + np.exp(-x))


def prep_params(inputs):
    """Small-parameter preprocessing shared by every core. Returns dict of
    numpy arrays keyed by kernel input name (excluding feats)."""
    f32 = np.float32
    a_e = _sigmoid(inputs["decay_e"].astype(np.float64)).astype(f32)
    a_i = _sigmoid(inputs["decay_i"].astype(np.float64)).astype(f32)
    b_e = ((1.0 - a_e) * inputs["w_e"]).astype(f32)
    b_i = ((1.0 - a_i) * inputs["w_i"]).astype(f32)

    # scan decay tiles with zeros at t==0 of each track (state reset)
    j = np.arange(FREE)
    boundary = (j % T == 0)
    mE = np.where(boundary[None, :], 0.0, a_e[:, None]).astype(f32)
    mI = np.where(boundary[None, :], 0.0, a_i[:, None]).astype(f32)
    masks = np.empty((128, 2 * NH * FREE), f32)  # cast below
    for h in range(NH):
        masks[:, h * FREE:(h + 1) * FREE] = mE[h * 128:(h + 1) * 128]
        masks[:, (NH + h) * FREE:(NH + h + 1) * FREE] = \
            mI[h * 128:(h + 1) * 128]

    cols = np.zeros((128, NCOLS), f32)

    def put_cols(name, vec, n):
        o = _col_off(name)
        for jj in range(n):
            cols[:, o + jj] = vec[jj * 128:(jj + 1) * 128]

    put_cols("bin", inputs["b_in"], NH)
    put_cols("bup", inputs["b_up"], NF)
    put_cols("bdn", inputs["b_dn"], NH)
    put_cols("l1b", inputs["ln1_b"], NH)
    put_cols("l2b", inputs["ln2_b"], NH)
    put_cols("be", b_e, NH)
    put_cols("bi", b_i, NH)

    bcat = np.concatenate(
        [inputs["b_delta"], inputs["b_vis"], inputs["b_track"]], axis=0)
    rows = np.concatenate(
        [inputs["ln1_s"], inputs["ln2_s"], bcat]).reshape(1, NROWS)

    def pack_win(w):
        # chunks at offsets [0,128,256,384,389]; rows [389:512) of the tail
        # chunk are zeroed (they overlap chunk 3)
        tail = np.zeros((128, H), w.dtype)
        tail[123:, :] = w[512:DIN, :]
        stack = [w[c * 128:(c + 1) * 128, :] for c in range(4)] + [tail]
        return np.ascontiguousarray(
            np.stack(stack, 1).reshape(128, KIN * H))

    def pack_k(w, nk):
        # (nk*128, M) -> (128, nk*M)
        M = w.shape[1]
        return np.ascontiguousarray(
            w.reshape(nk, 128, M).transpose(1, 0, 2).reshape(128, nk * M))

    wcat = np.concatenate(
        [inputs["W_delta"], inputs["W_vis"], inputs["W_track"]], axis=1)

    return {
        "masks": masks.astype(BF16),
        "cols": cols,
        "rows": rows.astype(BF16),
        "win": pack_win(inputs["W_in"]).astype(BF16),
        "wup": pack_k(inputs["W_up"], NH).astype(BF16),
        "wdn": pack_k(inputs["W_dn"], NF).astype(BF16),
        "wcat": pack_k(wcat, NH).astype(BF16),
    }


def prep_core_feats(inputs, c):
    """Per-core feature tensors in track-major / time-fastest token order.
    Returns (feats_bf16, track_f32)."""
    sl = slice(c * NPC, (c + 1) * NPC)

    def perm(x):
        return np.ascontiguousarray(
            x[:, :, sl, :].transpose(0, 2, 1, 3)).reshape(TOK, -1)

    track = perm(inputs["track_feats"])
    feats = np.empty((TOK, DIN), BF16)
    feats[:, 0:D] = track
    feats[:, D:D + DC] = perm(inputs["corr_feats"])
    feats[:, D + DC:D + DC + DF] = perm(inputs["flow_feats"])
    feats[:, D + DC + DF:DIN] = perm(inputs["vis"])
    return feats, np.ascontiguousarray(track, dtype=np.float32)


def unperm_core(out_c, nfeat):
    """(TOK, nfeat) track-major -> (B, T, NPC, nfeat)."""
    return out_c.reshape(B, NPC, T, nfeat).transpose(0, 2, 1, 3)


# ======================================================================
# Entry point
# ======================================================================

_CACHE = {}


def _build_nc():
    import concourse.bacc as bacc
    import concourse.tile as tile

    if "nc" in _CACHE:
        return _CACHE["nc"]

    nc = bacc.Bacc("TRN2", target_bir_lowering=False, debug=False)
    ins, outs = make_io(nc, TOK)
    with tile.TileContext(nc) as tc:
        build_kernel(tc, outs, ins, ntiles=NTILES)
    nc.compile()
    _CACHE["nc"] = nc
    return nc


def kernel(**inputs):
    from concourse.bass_utils import run_bass_kernel_spmd

    inputs = {k: np.asarray(v) for k, v in inputs.items()}
    params = prep_params(inputs)

    in_maps = []
    for c in range(NCORES):
        m = dict(params)
        m["feats"], m["trackres"] = prep_core_feats(inputs, c)
        in_maps.append(m)

    nc = _build_nc()
    res = run_bass_kernel_spmd(nc, in_maps, core_ids=list(range(NCORES)))

    delta = np.empty((B, T, N, 2), np.float32)
    vis_o = np.empty((B, T, N, 1), np.float32)
    track_o = np.empty((B, T, N, D), np.float32)
    for c in range(NCORES):
        sl = slice(c * NPC, (c + 1) * NPC)
        r = res.results[c]
        oc = unperm_core(r["ocat"], DOUT)
        delta[:, :, sl, :] = oc[:, :, :, 0:2]
        vis_o[:, :, sl, :] = oc[:, :, :, 2:3]
        track_o[:, :, sl, :] = oc[:, :, :, 3:]
    return (delta, vis_o, track_o)
